# revision 1
# baseline (speedup 1.0000x reference)
"""Trainium2 Bass kernel for nn_Attention (dense transformer attention block).

Reference computation (shapes fixed):
  x [2, 256, 48, 48] -> RMSNorm over channels -> 1x1 conv to qkv (8 heads, 64 dhead)
  -> prepend 4 learnable mem kv tokens -> softmax attention -> 1x1 conv out [2, 256, 48, 48]

Sharding: 8 cores = 2 batches x 4 head-pairs. Core c handles batch c//4 and
heads (2g, 2g+1), g = c%4. Each core computes its heads' attention and a
partial out-projection [256, 2304] in bf16; partials are ReduceScattered
per 512-column chunk within each batch's 4-core group; each core returns
its 64-channel slice and the host reassembles.

Key structure (v2):
  - RMSNorm sigma folded into the exp: k and v stay RAW; the per-key factor
    sigma_k enters as an exp bias column (ln sigma) and the denominator
    column of the attnv lhsT holds 1/sigma so den = sum exp(S) exactly.
  - exp split across engines: ACT handles 12 of 19 key tiles (table exp,
    bias column), DVE handles 7 via a Schraudolph bf16 bit-trick
    (tensor_scalar fp32->uint16, bitcast to bf16).
  - v produced directly transposed ([pos, dh]) by swapping matmul operands.
  - PE pipeline: sim(jt+1) emitted before attnv(jt) so the PE never queues
    behind an exp; psum: sim pool 2x[128,2,512] + acc pool 4x[128,512].
  - gpsimd queue carries ONLY partition broadcasts + collectives (ordered so
    nothing compute-critical queues behind a blocking collective); all DMAs
    ride sync/scalar; final out DMAs last.
"""
import math

import numpy as np

import concourse.mybir as mybir
import concourse.tile as tile
from concourse import bacc
from concourse.bass_utils import run_bass_kernel_spmd


F32 = mybir.dt.float32
F32R = mybir.dt.float32r
BF16 = mybir.dt.bfloat16
U16 = mybir.dt.uint16
EXP = mybir.ActivationFunctionType.Exp
SQRT = mybir.ActivationFunctionType.Sqrt
LN = mybir.ActivationFunctionType.Ln
SQUARE = mybir.ActivationFunctionType.Square
MULT = mybir.AluOpType.mult
ADD = mybir.AluOpType.add

DIM = 256
HEADS = 8
DHEAD = 64
MEM = 4
HID = 512
N = 48 * 48          # 2304 image positions
NJT = 18             # image key tiles of 128
GROUPS = [[0, 1, 2, 3], [4, 5, 6, 7]]

CHUNKS = [(0, 512), (512, 512), (1024, 512), (1536, 512), (2048, 256)]
NCH = len(CHUNKS)
DVE_JTS = frozenset({2, 4, 7, 9, 11, 14, 16, 17})  # interleaved with ACT

A_SCH = 128.0 / math.log(2.0)        # bf16 Schraudolph slope
B_SCH = 127.0 * 128.0 - 5.5          # minimax-centered magic constant


def _jt_slice(jt):
    """key tile jt -> (chunk index, col offset within chunk)."""
    pos0 = jt * 128
    for ci, (c0, cw) in enumerate(CHUNKS):
        if c0 <= pos0 < c0 + cw:
            return ci, pos0 - c0
    raise AssertionError(jt)


def build():
    nc = bacc.Bacc("TRN2", target_bir_lowering=False, debug=False,
                   enable_asserts=True, num_devices=8)
    x_d = nc.dram_tensor("x", [DIM, N], F32, kind="ExternalInput").ap()
    wqkv_d = nc.dram_tensor("wqkv", [DIM, 384], F32, kind="ExternalInput").ap()
    memk_d = nc.dram_tensor("memk", [128, MEM], F32, kind="ExternalInput").ap()
    memvst_d = nc.dram_tensor("memvst", [MEM, 2, 128], F32,
                              kind="ExternalInput").ap()
    woutT_d = nc.dram_tensor("woutT", [2, DHEAD, DIM], F32,
                             kind="ExternalInput").ap()
    out_d = nc.dram_tensor("out", [DHEAD, N], BF16, kind="ExternalOutput").ap()

    with tile.TileContext(nc) as tc:
        with (
            tc.tile_pool(name="consts", bufs=1) as consts,
            tc.tile_pool(name="big", bufs=1) as big,
            tc.tile_pool(name="io", bufs=2) as io,
            tc.tile_pool(name="wk", bufs=2) as wk,
            tc.tile_pool(name="ps_s", bufs=2, space="PSUM") as ps_s,
            tc.tile_pool(name="ps_a", bufs=4, space="PSUM") as ps_a,
            tc.tile_pool(name="dram", bufs=1, space="DRAM") as dram,
        ):
            # ---------------- input DMAs first (sync/scalar queues) --------
            # per-chunk tiles so chunk-0 compute starts as soon as its DMA
            # lands (whole-tile dependency granularity).
            xs = [[None] * NCH, [None] * NCH]
            dq = [nc.sync, nc.scalar, nc.gpsimd]
            for ci, (c0, cw) in enumerate(CHUNKS):
                for kt in range(2):
                    t = big.tile([128, 512], F32, tag=f"x{kt}_{ci}",
                                 name=f"x{kt}_{ci}")
                    xs[kt][ci] = t
                    dq[(2 * ci + kt) % 3].dma_start(
                        out=t[:, 0:cw],
                        in_=x_d[128 * kt:128 * kt + 128, c0:c0 + cw])
            wq_f = io.tile([128, 2, 384], F32, tag="wq_f")
            nc.sync.dma_start(out=wq_f[:, 0, :], in_=wqkv_d[0:128, :])
            nc.sync.dma_start(out=wq_f[:, 1, :], in_=wqkv_d[128:256, :])
            memk_f = io.tile([128, MEM], F32, tag="memk_f")
            nc.sync.dma_start(out=memk_f[:, :], in_=memk_d)
            memv_f = io.tile([MEM, 2, 128], F32, tag="memv_f")
            nc.sync.dma_start(out=memv_f[:, :, :], in_=memvst_d)
            woutA_f = io.tile([128, DIM], F32, tag="woutA_f")
            woutB_f = io.tile([128, DIM], F32, tag="woutB_f")
            nc.scalar.dma_start(out=woutA_f[64:128, :], in_=woutT_d[0, :, :])
            nc.scalar.dma_start(out=woutB_f[64:128, :], in_=woutT_d[1, :, :])

            # ---------------- constants / staging init --------------------
            ones_f = consts.tile([128, 1], F32)
            nc.vector.memset(ones_f[:, :], 1.0)
            ones_r = consts.tile([128, 128], F32R)
            nc.vector.tensor_copy(ones_r[:, :],
                                  ones_f[:, :].to_broadcast((128, 128)))

            # staging tiles for attnv lhsT: [keys, 2 heads, 128 cols]
            # cols: [0] = 1/sigma (denominator), [1:64] zeros, [64:128] = v^T
            vstag = [big.tile([128, 2, 128], BF16, tag=f"vst{jt}", name=f"vst{jt}")
                     for jt in range(NJT + 1)]
            for jt in range(NJT + 1):
                nc.gpsimd.memset(vstag[jt][:, :, :], 0.0)

            # per-key column tensors use duplicated columns: index 2*jt
            siginv = consts.tile([128, 2 * NJT], F32, tag="siginv")
            lnss = consts.tile([128, 2 * NJT], F32, tag="lnss")
            lnsig = consts.tile([128, 2 * NJT + 1], F32, tag="lnsig")
            nc.vector.memset(lnsig[:, 2 * NJT:2 * NJT + 1], 0.0)  # mem bias
            dve_bias = consts.tile([128, 2 * NJT], F32, tag="dve_bias")

            # ---------------- collective warmup (gpsimd only) -------------
            warm_sb = consts.tile([1, 32], F32)
            nc.vector.memset(warm_sb[:, :], 0.0)
            wi = dram.tile([1, 32], F32, tag="wi")
            wo = dram.tile([1, 32], F32, tag="wo")
            nc.scalar.dma_start(out=wi[:, :], in_=warm_sb[:, :])
            nc.gpsimd.collective_compute(
                "AllReduce", mybir.AluOpType.add,
                replica_groups=GROUPS,
                ins=[wi[:, :].opt()],
                outs=[wo[:, :].opt()],
            )

            # ---------------- weight conversions ---------------------------
            wq = consts.tile([128, 2, 384], F32R, tag="wq")
            nc.vector.tensor_copy(wq[:, :, :], wq_f[:, :, :])
            kmem = consts.tile([128, MEM], BF16, tag="kmem")
            nc.vector.tensor_copy(kmem[:, :], memk_f[:, :])
            nc.vector.tensor_copy(vstag[NJT][0:MEM, :, :], memv_f[:, :, :])
            woutA = consts.tile([128, DIM], BF16, tag="woutA")
            woutB = consts.tile([128, DIM], BF16, tag="woutB")
            nc.vector.tensor_copy(woutA[64:128, :], woutA_f[64:128, :])
            nc.vector.tensor_copy(woutB[64:128, :], woutB_f[64:128, :])
            wouts = [woutA, woutB]

            # ---------------- prep: RMS stats + qkv ------------------------
            qb = [None] * NCH
            kb = [None] * NCH
            sigc_b = consts.tile([128, 2 * NJT], BF16, tag="sigc_b")

            for ci, (c0, cw) in enumerate(CHUNKS):
                njs = cw // 128
                xsq = wk.tile([128, 2, 512], F32R, tag="xsq", name=f"xsq_{ci}")
                for kt in range(2):
                    nc.scalar.activation(xsq[:, kt, 0:cw],
                                         xs[kt][ci][:, 0:cw], SQUARE)
                ssq = ps_a.tile([128, 512], F32, tag="a", name=f"ssq_{ci}")
                for kt in range(2):
                    nc.tensor.matmul(ssq[:, 0:cw], ones_r[:, :],
                                     xsq[:, kt, 0:cw],
                                     start=(kt == 0), stop=(kt == 1))
                # rows: sigma = 16/l2 (sqrt then fast reciprocal)
                sinvr = big.tile([128, 512], F32, tag=f"sr{ci}", name=f"sr{ci}")
                nc.scalar.activation(sinvr[:, 0:cw], ssq[:, 0:cw], SQRT,
                                     scale=1.0 / 256.0)
                nc.vector.reciprocal_approx_fast(sinvr[:, 0:cw],
                                                 sinvr[:, 0:cw])
                # column-form sum of squares (per-position, on partitions).
                # fp32r moving FD must be even -> duplicated 2-wide columns.
                sgp = ps_a.tile([128, 512], F32, tag="a", name=f"sgp_{ci}")
                for js in range(njs):
                    for kt in range(2):
                        nc.tensor.matmul(
                            sgp[:, 2 * js:2 * js + 2],
                            xsq[:, kt, js * 128:js * 128 + 128],
                            ones_r[:, 0:2],
                            start=(kt == 0), stop=(kt == 1),
                        )
                # 1/sigma = l2/16 = sqrt(ssq/256), keys on partitions
                nc.scalar.activation(siginv[:, ci * 8:ci * 8 + 2 * njs],
                                     sgp[:, 0:2 * njs], SQRT,
                                     scale=1.0 / 256.0)
                # f32r copy of x for the qkv matmuls (consumer needs rounding)
                xrt = wk.tile([128, 2, 512], F32R, tag="xr", name=f"xr_{ci}")
                for kt in range(2):
                    nc.vector.tensor_copy(xrt[:, kt, 0:cw],
                                          xs[kt][ci][:, 0:cw])
                xr = [xrt[:, 0, :], xrt[:, 1, :]]
                # q, k projections (normalized at readout); v direct-transposed
                qk = ps_s.tile([128, 2, 512], F32, tag="s", name=f"qk_{ci}")
                for m in range(2):  # 0 = q, 1 = k
                    for kt in range(2):
                        nc.tensor.matmul(
                            qk[:, m, 0:cw],
                            wq[:, kt, m * 128:m * 128 + 128],
                            xr[kt][:, 0:cw],
                            start=(kt == 0), stop=(kt == 1),
                        )
                vps = ps_a.tile([128, 4, 2, 64], F32, tag="a", name=f"vps_{ci}")
                for js in range(njs):
                    for kt in range(2):
                        nc.tensor.matmul(
                            vps[:, js, :, :],
                            xr[kt][:, js * 128:js * 128 + 128],
                            wq[:, kt, 256:384],
                            start=(kt == 0), stop=(kt == 1),
                        )
                qb[ci] = big.tile([128, 512], BF16, tag=f"q{ci}", name=f"qb{ci}")
                kb[ci] = big.tile([128, 512], BF16, tag=f"k{ci}", name=f"kb{ci}")
                nc.vector.tensor_mul(qb[ci][:, 0:cw], qk[:, 0, 0:cw],
                                     sinvr[:, 0:cw])
                nc.vector.tensor_mul(kb[ci][:, 0:cw], qk[:, 1, 0:cw],
                                     sinvr[:, 0:cw])
                for js in range(njs):
                    jt = ci * 4 + js
                    nc.vector.tensor_copy(
                        vstag[jt][:, :, 64:128], vps[:, js, :, :])

            # ---------------- sigma columns (single Ln batch) --------------
            # siginv holds 1/sigma per key (duplicated cols). Staging col 0
            # needs 1/sigma (bf16); ACT exp bias needs ln(sigma) =
            # -ln(1/sigma); DVE exp needs B + A*ln(sigma).
            nc.vector.tensor_copy(sigc_b[:, :], siginv[:, :])
            nc.scalar.activation(lnss[:, :], siginv[:, :], LN)
            nc.vector.tensor_scalar(out=lnsig[:, 0:2 * NJT], in0=lnss[:, :],
                                    scalar1=-1.0, scalar2=None, op0=MULT)
            nc.vector.tensor_scalar(out=dve_bias[:, :], in0=lnss[:, :],
                                    scalar1=-A_SCH, scalar2=B_SCH,
                                    op0=MULT, op1=ADD)
            for jt in range(NJT):
                nc.vector.tensor_copy(
                    vstag[jt][:, :, 0:1],
                    sigc_b[:, 2 * jt:2 * jt + 1].to_broadcast((128, 2, 1)))

            # ---------------- attention ------------------------------------
            bis = [dram.tile([2, 128, cw], BF16, tag=f"bi{ci}", name=f"bi{ci}")
                   for ci, (c0, cw) in enumerate(CHUNKS)]
            bos = [dram.tile([DHEAD, cw], BF16, tag=f"bo{ci}", name=f"bo{ci}")
                   for ci, (c0, cw) in enumerate(CHUNKS)]
            accs_by_ci = [None] * NCH
            fin = {}

            def emit_sim(ci, jt):
                c0, cw = CHUNKS[ci]
                s_ps = ps_s.tile([128, 2, 512], F32, tag="s",
                                 name=f"s_{ci}_{jt}")
                if jt < NJT:
                    km = 128
                    kc, off = _jt_slice(jt)
                    klhs = [kb[kc][64 * h:64 * h + 64, off:off + 128]
                            for h in range(2)]
                else:
                    km = MEM
                    klhs = [kmem[64 * h:64 * h + 64, :] for h in range(2)]
                for h in range(2):
                    nc.tensor.matmul(
                        s_ps[0:km, h, 0:cw],
                        klhs[h],
                        qb[ci][64 * h:64 * h + 64, 0:cw],
                        start=True, stop=True,
                    )
                return s_ps, km

            def emit_exp(ci, jt, s_ps, km):
                c0, cw = CHUNKS[ci]
                if jt in DVE_JTS:
                    P = wk.tile([128, 2, 512], U16, tag="Pd",
                                name=f"Pd_{ci}_{jt}")
                    nc.vector.tensor_scalar(
                        out=P[:, :, 0:cw], in0=s_ps[:, :, 0:cw],
                        scalar1=A_SCH, scalar2=dve_bias[:, 2 * jt:2 * jt + 1],
                        op0=MULT, op1=ADD)
                    return P, True
                P = wk.tile([128, 2, 512], BF16, tag="Pa", name=f"Pa_{ci}_{jt}")
                nc.scalar.activation(P[0:km, :, 0:cw], s_ps[0:km, :, 0:cw],
                                     EXP, bias=lnsig[0:km, 2 * jt:2 * jt + 1])
                return P, False

            def emit_attnv(ci, jt, P, km, cast):
                c0, cw = CHUNKS[ci]
                accs = accs_by_ci[ci]
                for h in range(2):
                    rhs = P[0:km, h, 0:cw]
                    if cast:
                        rhs = rhs.bitcast(BF16)
                    nc.tensor.matmul(
                        accs[h][:, 0:cw],
                        vstag[jt][0:km, h, :],
                        rhs,
                        start=(jt == 0), stop=(jt == NJT),
                        skip_group_check=True,
                    )

            def emit_fin_a(ci):
                """recip + partition broadcasts for chunk ci (DVE+gpsimd)."""
                c0, cw = CHUNKS[ci]
                accs = accs_by_ci[ci]
                rec = wk.tile([128, 2, 512], F32, tag="rec", name=f"rec_{ci}")
                rb = wk.tile([128, 2, 512], F32, tag="rb", name=f"rb_{ci}")
                for h in range(2):
                    nc.vector.reciprocal_approx_fast(
                        rec[0:1, h, 0:cw], accs[h][0:1, 0:cw])
                    nc.gpsimd.partition_broadcast(rb[:, h, 0:cw],
                                                  rec[0:1, h, 0:cw])
                fin[ci] = (rec, rb)

            def emit_fin_b(ci):
                """normalize oT (DVE)."""
                c0, cw = CHUNKS[ci]
                accs = accs_by_ci[ci]
                rec, rb = fin[ci]
                oT = wk.tile([128, 2, 512], BF16, tag="oT", name=f"oT_{ci}")
                for h in range(2):
                    nc.vector.tensor_mul(oT[64:128, h, 0:cw],
                                         accs[h][64:128, 0:cw],
                                         rb[64:128, h, 0:cw])
                fin[ci] = oT

            def emit_fin_c(ci):
                """out-projection (PE)."""
                c0, cw = CHUNKS[ci]
                oT = fin[ci]
                op = ps_s.tile([128, 2, 512], F32, tag="s", name=f"op_{ci}")
                for mt in range(2):
                    for h in range(2):
                        nc.tensor.matmul(
                            op[:, mt, 0:cw],
                            wouts[h][64:128, mt * 128:mt * 128 + 128],
                            oT[64:128, h, 0:cw],
                            start=(h == 0), stop=(h == 1),
                        )
                fin[ci] = op

            def emit_fin_d(ci):
                """osb copy (DVE) + DMA to DRAM (sync) + RS (gpsimd)."""
                c0, cw = CHUNKS[ci]
                op = fin[ci]
                osb = wk.tile([128, 2, 512], BF16, tag="osb", name=f"osb_{ci}")
                nc.vector.tensor_copy(osb[:, :, 0:cw], op[:, :, 0:cw])
                for mt in range(2):
                    nc.sync.dma_start(out=bis[ci][mt, :, :],
                                      in_=osb[:, mt, 0:cw])
                nc.gpsimd.collective_compute(
                    "ReduceScatter", mybir.AluOpType.add,
                    replica_groups=GROUPS,
                    ins=[bis[ci][:, :, :].opt()],
                    outs=[bos[ci][:, :].opt()],
                )

            for ci, (c0, cw) in enumerate(CHUNKS):
                acc0 = ps_a.tile([128, 512], F32, tag="a", name=f"acc0_{ci}")
                acc1 = ps_a.tile([128, 512], F32, tag="a", name=f"acc1_{ci}")
                accs_by_ci[ci] = [acc0, acc1]
                pend = None
                for jt in range(NJT + 1):
                    s_ps, km = emit_sim(ci, jt)
                    if pend is not None:
                        emit_attnv(ci, *pend)
                    # finish hooks for the previous chunk BEFORE this jt's
                    # exp: a hook's DVE work must not queue behind an exp
                    # that transitively waits on a psum buffer the hook
                    # itself frees (deadlock otherwise).
                    if ci > 0:
                        if jt == 3:
                            emit_fin_a(ci - 1)
                        elif jt == 5:
                            emit_fin_b(ci - 1)
                        elif jt == 8:
                            emit_fin_c(ci - 1)
                        elif jt == 10:
                            emit_fin_d(ci - 1)
                    P, cast = emit_exp(ci, jt, s_ps, km)
                    pend = (jt, P, km, cast)
                emit_attnv(ci, *pend)
            emit_fin_a(NCH - 1)
            emit_fin_b(NCH - 1)
            emit_fin_c(NCH - 1)
            emit_fin_d(NCH - 1)
            for ci, (c0, cw) in enumerate(CHUNKS):
                nc.sync.dma_start(out=out_d[:, c0:c0 + cw], in_=bos[ci][:, :])
    nc.compile()
    return nc


_NC = None
_last_in_maps = None


def _get_nc():
    global _NC
    if _NC is None:
        _NC = build()
    return _NC


def make_in_maps(x, gamma, mem_kv, w_qkv, w_out):
    x = np.asarray(x, np.float32)
    gamma = np.asarray(gamma, np.float32).reshape(DIM)
    mem_kv = np.asarray(mem_kv, np.float32)
    w_qkv = np.asarray(w_qkv, np.float32)
    w_out = np.asarray(w_out, np.float32)

    g1 = 1.0 + gamma  # [256]
    scale = DHEAD ** -0.5
    in_maps = []
    for core in range(8):
        b, g = core // 4, core % 4
        hA, hB = 2 * g, 2 * g + 1
        blocks = []
        for t in range(3):  # q, k, v
            for h in (hA, hB):
                wblk = w_qkv[t * HID + h * DHEAD: t * HID + (h + 1) * DHEAD, :]
                if t == 0:
                    wblk = wblk * scale
                blocks.append(wblk.T)  # [256, 64]
        wqkvT = np.concatenate(blocks, axis=1) * g1[:, None]  # [256, 384]
        memk = np.concatenate(
            [mem_kv[0, hA].T, mem_kv[0, hB].T], axis=0)  # [128, 4]
        # mem staging: [4, 2, 128] = [1/sigma(=1) | zeros(63) | v(64)]
        memvst = np.zeros((MEM, 2, 128), np.float32)
        memvst[:, :, 0] = 1.0
        memvst[:, 0, 64:128] = mem_kv[1, hA]
        memvst[:, 1, 64:128] = mem_kv[1, hB]
        woutT = np.stack(
            [w_out[:, hA * DHEAD:(hA + 1) * DHEAD].T,
             w_out[:, hB * DHEAD:(hB + 1) * DHEAD].T], axis=0)  # [2, 64, 256]
        in_maps.append({
            "x": np.ascontiguousarray(x[b].reshape(DIM, N)),
            "wqkv": np.ascontiguousarray(wqkvT),
            "memk": np.ascontiguousarray(memk),
            "memvst": np.ascontiguousarray(memvst),
            "woutT": np.ascontiguousarray(woutT),
        })
    return in_maps


def kernel(x, gamma, mem_kv, w_qkv, w_out):
    global _last_in_maps
    in_maps = make_in_maps(x, gamma, mem_kv, w_qkv, w_out)
    _last_in_maps = in_maps
    nc = _get_nc()
    res = run_bass_kernel_spmd(nc, in_maps, core_ids=list(range(8)))
    out = np.empty((2, DIM, N), np.float32)
    for core in range(8):
        b, g = core // 4, core % 4
        out[b, 64 * g:64 * g + 64, :] = np.asarray(
            res.results[core]["out"], dtype=np.float32)
    return out.reshape(2, DIM, 48, 48)



# revision 16
# speedup vs baseline: 1.0389x; 1.0389x over previous
"""Trainium2 Bass kernel for nn_Attention (dense transformer attention block).

Reference computation (shapes fixed):
  x [2, 256, 48, 48] -> RMSNorm over channels -> 1x1 conv to qkv (8 heads, 64 dhead)
  -> prepend 4 learnable mem kv tokens -> softmax attention -> 1x1 conv out [2, 256, 48, 48]

Sharding (v3, query-sharded, collective-free): 8 cores = 2 batches x 4
query-quarters. Core c handles batch c//4 and queries [576*(c%4), 576*(c%4+1)).
Each core computes k/v for ALL 8 heads over all 2304 positions, q for its own
576 queries, runs full attention for all heads on its query slice, and
finishes the w_out projection on-core (contraction over all 512 hidden dims is
local) -> writes its [256, 576] output slice. No collectives at all.

Key structure:
  - x is L2-normalized ONCE (xn = x * sigma, sigma=16/l2 per position) before
    the qkv projections, so q, k, v all come out normalized: no per-key exp
    bias, no sigma column machinery; the attnv denominator column is exactly
    1.0 (tiny memset).
  - attnv lhsT staging per key tile: [keys, 8 heads, 68] with cols 0:64 = v^T
    and col 64 = 1.0 (denominator); M=65 stationary operand -> fast
    LDWEIGHTS, no zero padding. acc rows 0:64 = attention out, row 64 = den.
  - exp split across ACT (table exp) and DVE (Schraudolph bf16 bit-trick);
    per-jt assignment in DVE_JTS.
  - PE pipeline: sim(jt+1) emitted before attnv(jt); sim head pairs run
    concurrently on disjoint PE row groups (K=64 at base partitions 0/64).
  - engine placement: all input DMAs on sync+gpsimd queues (never on ACT/DVE);
    gpsimd = xn muls + partition broadcasts; ACT = squares/sqrts/k+q casts +
    exp share; DVE = weight converts, v-staging, oT normalize, osb, recips +
    exp share.
"""
import math

import numpy as np

import concourse.mybir as mybir
import concourse.tile as tile
from concourse import bacc
from concourse.bass_utils import run_bass_kernel_spmd


F32 = mybir.dt.float32
F32R = mybir.dt.float32r
BF16 = mybir.dt.bfloat16
U16 = mybir.dt.uint16
EXP = mybir.ActivationFunctionType.Exp
SQRT = mybir.ActivationFunctionType.Sqrt
SQUARE = mybir.ActivationFunctionType.Square
COPY = mybir.ActivationFunctionType.Copy
MULT = mybir.AluOpType.mult
ADD = mybir.AluOpType.add

DIM = 256
HEADS = 8
DHEAD = 64
MEM = 4
HID = 512
N = 48 * 48          # 2304 image positions
NJT = 18             # image key tiles of 128
QTOT = 576           # queries per core
QC = 288             # query chunk (2 chunks per core)
NHP = 4              # head pairs
VW = 128             # staging width per head (den col 0, zeros, v 64:128)

CHUNKS = [(0, 512), (512, 512), (1024, 512), (1536, 512), (2048, 256)]
NCH = len(CHUNKS)
DVE_JTS = frozenset({1, 3, 5, 7, 9, 11, 13, 15, 17})  # interleaved with ACT

A_SCH = 128.0 / math.log(2.0)        # bf16 Schraudolph slope
B_SCH = 127.0 * 128.0 - 5.5          # minimax-centered magic constant


def build():
    nc = bacc.Bacc("TRN2", target_bir_lowering=False, debug=False,
                   enable_asserts=True, num_devices=8)
    x_d = nc.dram_tensor("x", [DIM, N], F32, kind="ExternalInput").ap()
    xq_d = nc.dram_tensor("xq", [DIM, QTOT], F32, kind="ExternalInput").ap()
    wqkv_d = nc.dram_tensor("wqkv", [DIM, 3 * HID], F32,
                            kind="ExternalInput").ap()
    memk_d = nc.dram_tensor("memk", [128, 4 * NHP], F32,
                            kind="ExternalInput").ap()
    memvst_d = nc.dram_tensor("memvst", [MEM, HEADS, VW], F32,
                              kind="ExternalInput").ap()
    woutT_d = nc.dram_tensor("woutT", [DHEAD, HEADS, DIM], F32,
                             kind="ExternalInput").ap()
    out_d = nc.dram_tensor("out", [DIM, QTOT], BF16, kind="ExternalOutput").ap()

    with tile.TileContext(nc) as tc:
        with (
            tc.tile_pool(name="consts", bufs=1) as consts,
            tc.tile_pool(name="big", bufs=1) as big,
            tc.tile_pool(name="io", bufs=1) as io,
            tc.tile_pool(name="wk", bufs=2) as wk,
            tc.tile_pool(name="ps_s", bufs=2, space="PSUM") as ps_s,
            tc.tile_pool(name="ps_a", bufs=4, space="PSUM") as ps_a,
        ):
            # ---------------- input DMAs (sync + gpsimd queues only) -------
            xs = [[None] * NCH, [None] * NCH]
            for ci, (c0, cw) in enumerate(CHUNKS):
                for kt in range(2):
                    xs[kt][ci] = big.tile([128, 512], F32, tag=f"x{kt}_{ci}",
                                          name=f"x{kt}_{ci}")
            wq_f = io.tile([128, 2, 3 * HID], F32, tag="wq_f")
            xq_f = io.tile([128, 2, QTOT], F32, tag="xq_f")
            wout_f = io.tile([128, HEADS, DIM], F32, tag="wout_f")
            memk_f = io.tile([128, 4 * NHP], F32, tag="memk_f")
            memvst_f = io.tile([MEM, HEADS, VW], F32, tag="memv_f")

            # sync queue: chunk-0 x, k-weights, then the rest of x + consts
            nc.sync.dma_start(out=xs[0][0][:, :], in_=x_d[0:128, 0:512])
            nc.sync.dma_start(out=xs[1][0][:, :], in_=x_d[128:256, 0:512])
            nc.sync.dma_start(out=wq_f[:, 0, 512:1024],
                              in_=wqkv_d[0:128, 512:1024])
            nc.sync.dma_start(out=wq_f[:, 1, 512:1024],
                              in_=wqkv_d[128:256, 512:1024])
            for ci, (c0, cw) in enumerate(CHUNKS[1:], start=1):
                nc.sync.dma_start(out=xs[0][ci][:, 0:cw],
                                  in_=x_d[0:128, c0:c0 + cw])
                nc.sync.dma_start(out=xs[1][ci][:, 0:cw],
                                  in_=x_d[128:256, c0:c0 + cw])
            nc.sync.dma_start(out=memk_f[:, :], in_=memk_d)
            nc.sync.dma_start(out=memvst_f[:, :, :], in_=memvst_d)
            nc.sync.dma_start(out=wout_f[64:128, :, :], in_=woutT_d)
            # gpsimd queue: v-weights, xq, q-weights (then xn muls later)
            nc.gpsimd.dma_start(out=wq_f[:, 0, 1024:1536],
                                in_=wqkv_d[0:128, 1024:1536])
            nc.gpsimd.dma_start(out=wq_f[:, 1, 1024:1536],
                                in_=wqkv_d[128:256, 1024:1536])
            nc.gpsimd.dma_start(out=xq_f[:, 0, :], in_=xq_d[0:128, :])
            nc.gpsimd.dma_start(out=xq_f[:, 1, :], in_=xq_d[128:256, :])
            nc.gpsimd.dma_start(out=wq_f[:, 0, 0:512], in_=wqkv_d[0:128, 0:512])
            nc.gpsimd.dma_start(out=wq_f[:, 1, 0:512],
                                in_=wqkv_d[128:256, 0:512])

            # ---------------- constants / staging init --------------------
            ones_f = consts.tile([128, 1], F32)
            nc.vector.memset(ones_f[:, :], 1.0)
            ones_r = consts.tile([128, 128], F32R)
            nc.vector.tensor_copy(ones_r[:, :],
                                  ones_f[:, :].to_broadcast((128, 128)))

            # attnv lhsT staging: [keys, 8 heads, VW]
            # col 0 = 1.0 (denominator), cols 1:64 zeros, cols 64:128 = v^T
            vstag = [big.tile([128, HEADS, VW], BF16, tag=f"vst{jt}",
                              name=f"vst{jt}")
                     for jt in range(NJT + 1)]
            for jt in range(NJT):
                nc.vector.memset(vstag[jt][:, :, 0:1], 1.0)

            # weight conversions (DVE); k block first (needed earliest)
            wq = consts.tile([128, 2, 3 * HID], F32R, tag="wq")
            nc.vector.tensor_copy(wq[:, :, 512:1024], wq_f[:, :, 512:1024])
            nc.vector.tensor_copy(wq[:, :, 1024:1536], wq_f[:, :, 1024:1536])
            nc.vector.tensor_copy(wq[:, :, 0:512], wq_f[:, :, 0:512])

            kb = [big.tile([128, N], BF16, tag=f"k{hp}", name=f"kb{hp}")
                  for hp in range(NHP)]
            qb = [big.tile([128, 2, QC], BF16, tag=f"q{hp}", name=f"qb{hp}")
                  for hp in range(NHP)]

            # ---------------- prep: per-chunk k/v --------------------------
            def prep_chunk(ci):
                c0, cw = CHUNKS[ci]
                njs = cw // 128
                xsq = wk.tile([128, 2, 512], F32R, tag="xsq", name=f"xsq{ci}")
                for kt in range(2):
                    nc.scalar.activation(xsq[:, kt, 0:cw],
                                         xs[kt][ci][:, 0:cw], SQUARE)
                ssq = ps_a.tile([128, 512], F32, tag="a", name=f"ssq{ci}")
                for kt in range(2):
                    nc.tensor.matmul(ssq[:, 0:cw], ones_r[:, :],
                                     xsq[:, kt, 0:cw],
                                     start=(kt == 0), stop=(kt == 1))
                # sigma rows = 16/l2 (sqrt then fast reciprocal)
                srow = wk.tile([128, 512], F32, tag="sr", name=f"sr{ci}")
                nc.scalar.activation(srow[:, 0:cw], ssq[:, 0:cw], SQRT,
                                     scale=1.0 / 256.0)
                nc.vector.reciprocal_approx_fast(srow[:, 0:cw], srow[:, 0:cw])
                # xn = x * sigma (normalized x, f32r for the matmuls)
                xn = wk.tile([128, 2, 512], F32R, tag="xn", name=f"xn{ci}")
                for kt in range(2):
                    nc.gpsimd.tensor_mul(xn[:, kt, 0:cw], xs[kt][ci][:, 0:cw],
                                         srow[:, 0:cw])
                # k for all 4 head pairs
                for hp in range(NHP):
                    kps = ps_a.tile([128, 512], F32, tag="a",
                                    name=f"kps{ci}_{hp}")
                    for kt in range(2):
                        nc.tensor.matmul(
                            kps[:, 0:cw],
                            wq[:, kt, 512 + 128 * hp:512 + 128 * hp + 128],
                            xn[:, kt, 0:cw],
                            start=(kt == 0), stop=(kt == 1))
                    nc.scalar.activation(kb[hp][:, c0:c0 + cw], kps[:, 0:cw],
                                         COPY)
                # v, directly transposed: [pos, 8 heads, 64]
                for js in range(njs):
                    jt = ci * 4 + js
                    nc.gpsimd.memset(vstag[jt][:, :, 1:64], 0.0)
                    vps = ps_a.tile([128, HEADS, DHEAD], F32, tag="a",
                                    name=f"vps{jt}")
                    for kt in range(2):
                        nc.tensor.matmul(
                            vps[:, :, :],
                            xn[:, kt, js * 128:js * 128 + 128],
                            wq[:, kt, 1024:1536],
                            start=(kt == 0), stop=(kt == 1))
                    nc.vector.tensor_copy(vstag[jt][:, :, 64:128],
                                          vps[:, :, :])

            # ---------------- prep: q path ---------------------------------
            def prep_q():
                xqsq = wk.tile([128, 2, QTOT], F32R, tag="xqsq")
                for kt in range(2):
                    nc.scalar.activation(xqsq[:, kt, :], xq_f[:, kt, :],
                                         SQUARE)
                sq_ps = ps_s.tile([128, 2, 512], F32, tag="s", name="sq_ps")
                for half in range(2):
                    for kt in range(2):
                        nc.tensor.matmul(
                            sq_ps[:, half, 0:QC], ones_r[:, :],
                            xqsq[:, kt, QC * half:QC * half + QC],
                            start=(kt == 0), stop=(kt == 1))
                sqrow = wk.tile([128, QTOT], F32, tag="sqr")
                for half in range(2):
                    nc.scalar.activation(sqrow[:, QC * half:QC * half + QC],
                                         sq_ps[:, half, 0:QC], SQRT,
                                         scale=1.0 / 256.0)
                nc.vector.reciprocal_approx_fast(sqrow[:, :], sqrow[:, :])
                xnq = consts.tile([128, 2, QTOT], F32R, tag="xnq")
                for kt in range(2):
                    nc.gpsimd.tensor_mul(xnq[:, kt, :], xq_f[:, kt, :],
                                         sqrow[:, :])
                for hp in range(NHP):
                    qps = ps_s.tile([128, 2, 512], F32, tag="s",
                                    name=f"qps{hp}")
                    for qc in range(2):
                        for kt in range(2):
                            nc.tensor.matmul(
                                qps[:, qc, 0:QC],
                                wq[:, kt, 128 * hp:128 * hp + 128],
                                xnq[:, kt, QC * qc:QC * qc + QC],
                                start=(kt == 0), stop=(kt == 1))
                    nc.scalar.activation(qb[hp][:, :, :], qps[:, :, 0:QC],
                                         COPY)

            prep_chunk(0)
            prep_chunk(1)
            # late consts (off the critical DVE path at start)
            kmem = consts.tile([128, 4 * NHP], BF16, tag="kmem")
            nc.vector.tensor_copy(kmem[:, :], memk_f[:, :])
            nc.vector.tensor_copy(vstag[NJT][0:MEM, :, :], memvst_f[:, :, :])
            wout_t = consts.tile([128, HEADS, DIM], BF16, tag="wout_t")
            nc.vector.tensor_copy(wout_t[64:128, :, :], wout_f[64:128, :, :])
            prep_q()
            prep_chunk(2)
            prep_chunk(3)
            prep_chunk(4)

            # ---------------- attention rounds -----------------------------
            # round r: qc = r // 4, hp = r % 4
            accs_by_r = {}
            oTs = [None] * NHP
            fin = {}

            def emit_sim(r, jt):
                hp = r % NHP
                qc = r // NHP
                s_ps = ps_s.tile([128, 2, 512], F32, tag="s",
                                 name=f"s_{r}_{jt}")
                if jt < NJT:
                    km = 128
                    klhs = [kb[hp][64 * h:64 * h + 64,
                                   128 * jt:128 * jt + 128]
                            for h in range(2)]
                else:
                    km = MEM
                    klhs = [kmem[64 * h:64 * h + 64, 4 * hp:4 * hp + MEM]
                            for h in range(2)]
                for h in range(2):
                    nc.tensor.matmul(
                        s_ps[0:km, h, 0:QC],
                        klhs[h],
                        qb[hp][64 * h:64 * h + 64, qc, :],
                        start=True, stop=True)
                return s_ps, km

            def emit_exp(r, jt, s_ps, km):
                if jt in DVE_JTS:
                    P = wk.tile([128, 2, QC], U16, tag="Pd",
                                name=f"Pd_{r}_{jt}")
                    nc.vector.tensor_scalar(
                        out=P[:, :, :], in0=s_ps[:, :, 0:QC],
                        scalar1=A_SCH, scalar2=B_SCH,
                        op0=MULT, op1=ADD)
                    return P, True
                P = wk.tile([128, 2, QC], BF16, tag="Pa", name=f"Pa_{r}_{jt}")
                nc.scalar.activation(P[0:km, :, :], s_ps[0:km, :, 0:QC], EXP)
                return P, False

            def emit_attnv(r, jt, P, km, cast):
                hp = r % NHP
                accs = accs_by_r[r]
                for h in range(2):
                    rhs = P[0:km, h, :]
                    if cast:
                        rhs = rhs.bitcast(BF16)
                    nc.tensor.matmul(
                        accs[h][:, 0:QC],
                        vstag[jt][0:km, 2 * hp + h, :],
                        rhs,
                        start=(jt == 0), stop=(jt == NJT),
                        skip_group_check=True)

            def emit_fin_a(r):
                """den reciprocal + partition broadcasts (DVE+gpsimd)."""
                accs = accs_by_r[r]
                rec = wk.tile([128, 2, QC], F32, tag="rec", name=f"rec{r}")
                rb = wk.tile([128, 2, QC], F32, tag="rb", name=f"rb{r}")
                for h in range(2):
                    nc.vector.reciprocal_approx_fast(
                        rec[0:1, h, :], accs[h][0:1, 0:QC])
                    nc.gpsimd.partition_broadcast(rb[:, h, :],
                                                  rec[0:1, h, :])
                fin[r] = rb

            def emit_fin_b(r):
                """normalize oT (DVE)."""
                hp = r % NHP
                accs = accs_by_r[r]
                rb = fin.pop(r)
                oT = big.tile([128, 2, QC], BF16, tag=f"oT{hp}",
                              name=f"oT_{r}")
                for h in range(2):
                    nc.vector.tensor_mul(oT[64:128, h, :],
                                         accs[h][64:128, 0:QC],
                                         rb[64:128, h, :])
                oTs[hp] = oT

            def emit_fin_c(qc):
                """out-projection over all 8 heads (PE)."""
                op = ps_s.tile([128, 2, 512], F32, tag="s", name=f"op{qc}")
                for mt in range(2):
                    for h in range(HEADS):
                        nc.tensor.matmul(
                            op[:, mt, 0:QC],
                            wout_t[64:128, h, 128 * mt:128 * mt + 128],
                            oTs[h // 2][64:128, h % 2, :],
                            start=(h == 0), stop=(h == HEADS - 1))
                fin[("op", qc)] = op

            def emit_fin_d(qc):
                """osb cast (DVE) + output DMA (sync)."""
                op = fin.pop(("op", qc))
                osb = wk.tile([128, 2, QC], BF16, tag="osb", name=f"osb{qc}")
                nc.vector.tensor_copy(osb[:, :, :], op[:, :, 0:QC])
                for mt in range(2):
                    nc.sync.dma_start(
                        out=out_d[128 * mt:128 * mt + 128,
                                  QC * qc:QC * qc + QC],
                        in_=osb[:, mt, :])

            def hooks(r, jt):
                prev = r - 1
                if prev < 0:
                    return
                if jt == 3:
                    emit_fin_a(prev)
                elif jt == 5:
                    emit_fin_b(prev)
                elif jt == 8 and prev % NHP == NHP - 1:
                    emit_fin_c(prev // NHP)
                elif jt == 10 and prev % NHP == NHP - 1:
                    emit_fin_d(prev // NHP)

            for r in range(2 * NHP):
                acc0 = ps_a.tile([128, 512], F32, tag="a", name=f"acc0_{r}")
                acc1 = ps_a.tile([128, 512], F32, tag="a", name=f"acc1_{r}")
                accs_by_r[r] = [acc0, acc1]
                pend = None
                for jt in range(NJT + 1):
                    s_ps, km = emit_sim(r, jt)
                    if pend is not None:
                        emit_attnv(r, *pend)
                    hooks(r, jt)
                    P, cast = emit_exp(r, jt, s_ps, km)
                    pend = (jt, P, km, cast)
                emit_attnv(r, *pend)
            last = 2 * NHP - 1
            emit_fin_a(last)
            emit_fin_b(last)
            emit_fin_c(1)
            emit_fin_d(1)
    nc.compile()
    return nc


_NC = None
_last_in_maps = None


def _get_nc():
    global _NC
    if _NC is None:
        _NC = build()
    return _NC


def make_in_maps(x, gamma, mem_kv, w_qkv, w_out):
    x = np.asarray(x, np.float32)
    gamma = np.asarray(gamma, np.float32).reshape(DIM)
    mem_kv = np.asarray(mem_kv, np.float32)
    w_qkv = np.asarray(w_qkv, np.float32)
    w_out = np.asarray(w_out, np.float32)

    g1 = 1.0 + gamma  # [256]
    scale = DHEAD ** -0.5
    # wqkvT [256, 1536]: cols [q 8x64 (scaled) | k 8x64 | v 8x64], x g1 rows
    blocks = []
    for t in range(3):
        for h in range(HEADS):
            wblk = w_qkv[t * HID + h * DHEAD: t * HID + (h + 1) * DHEAD, :]
            if t == 0:
                wblk = wblk * scale
            blocks.append(wblk.T)  # [256, 64]
    wqkvT = np.concatenate(blocks, axis=1) * g1[:, None]  # [256, 1536]

    # memk [128, 16]: pair hp at cols 4hp; rows 0:64 head 2hp, 64:128 2hp+1
    memk = np.zeros((128, 4 * NHP), np.float32)
    for hp in range(NHP):
        memk[0:64, 4 * hp:4 * hp + 4] = mem_kv[0, 2 * hp].T
        memk[64:128, 4 * hp:4 * hp + 4] = mem_kv[0, 2 * hp + 1].T

    # memvst [4, 8, VW]: col 0 = 1.0 (den), cols 64:128 = mem_v
    memvst = np.zeros((MEM, HEADS, VW), np.float32)
    memvst[:, :, 0] = 1.0
    for h in range(HEADS):
        memvst[:, h, 64:128] = mem_kv[1, h]

    # woutT [64, 8, 256]: [d, h, o] = w_out[o, 64h+d]
    woutT = np.ascontiguousarray(
        w_out.T.reshape(HEADS, DHEAD, DIM).transpose(1, 0, 2))

    shared = {
        "wqkv": np.ascontiguousarray(wqkvT),
        "memk": np.ascontiguousarray(memk),
        "memvst": np.ascontiguousarray(memvst),
        "woutT": np.ascontiguousarray(woutT),
    }
    in_maps = []
    for core in range(8):
        b, qp = core // 4, core % 4
        xb = np.ascontiguousarray(x[b].reshape(DIM, N))
        m = dict(shared)
        m["x"] = xb
        m["xq"] = np.ascontiguousarray(xb[:, qp * QTOT:(qp + 1) * QTOT])
        in_maps.append(m)
    return in_maps


def kernel(x, gamma, mem_kv, w_qkv, w_out):
    global _last_in_maps
    in_maps = make_in_maps(x, gamma, mem_kv, w_qkv, w_out)
    _last_in_maps = in_maps
    nc = _get_nc()
    res = run_bass_kernel_spmd(nc, in_maps, core_ids=list(range(8)))
    out = np.empty((2, DIM, N), np.float32)
    for core in range(8):
        b, qp = core // 4, core % 4
        out[b, :, qp * QTOT:(qp + 1) * QTOT] = np.asarray(
            res.results[core]["out"], dtype=np.float32)
    return out.reshape(2, DIM, 48, 48)


# revision 24
# speedup vs baseline: 1.2413x; 1.1948x over previous
"""Trainium2 Bass kernel for nn_Attention (dense transformer attention block).

Reference computation (shapes fixed):
  x [2, 256, 48, 48] -> RMSNorm over channels -> 1x1 conv to qkv (8 heads, 64 dhead)
  -> prepend 4 learnable mem kv tokens -> softmax attention -> 1x1 conv out [2, 256, 48, 48]

Sharding (v4, head-sharded, collective-free): 8 cores = 2 batches x 4
head-pairs. Core c handles batch c//4 and heads (2g, 2g+1), g = c%4. Each core
runs its heads' attention over ALL 2304 queries (512-wide chunks keep the PE
at peak column rate) and emits its PARTIAL out-projection [256, 2304] in f32,
DMA'd straight from PSUM. The host sums the 4 partials per batch -- no
on-device collective at all.

Key structure:
  - x is L2-normalized ONCE (xn = x * sigma, sigma=16/l2 per position, bf16)
    before the qkv projections, so q, k, v all come out normalized: no per-key
    exp bias, no sigma column machinery; the attnv denominator column is
    exactly 1.0.
  - attnv lhsT staging per key tile: [keys, 2 heads, 128], col 0 = 1.0 (den),
    cols 64:128 = v^T; acc row 0 = denominator, rows 64:128 = attention out.
  - exp split across ACT (table exp) and DVE (Schraudolph bf16 bit-trick).
  - PE pipeline: sim(jt+1) emitted before attnv(jt); sim head pairs run
    concurrently on disjoint PE row groups (K=64 at base partitions 0/64).
  - engine placement: all input DMAs on the sync queue; gpsimd = xn muls +
    staging zero-memsets + partition broadcasts; ACT = squares/sqrts/qb cast +
    exp share; DVE = kb/v staging, oT normalize, recips + exp share.
"""
import math

import numpy as np

import concourse.mybir as mybir
import concourse.tile as tile
from concourse import bacc
from concourse.bass_utils import run_bass_kernel_spmd


F32 = mybir.dt.float32
F32R = mybir.dt.float32r
BF16 = mybir.dt.bfloat16
U16 = mybir.dt.uint16
EXP = mybir.ActivationFunctionType.Exp
SQRT = mybir.ActivationFunctionType.Sqrt
SQUARE = mybir.ActivationFunctionType.Square
COPY = mybir.ActivationFunctionType.Copy
MULT = mybir.AluOpType.mult
ADD = mybir.AluOpType.add

DIM = 256
HEADS = 8
DHEAD = 64
MEM = 4
HID = 512
N = 48 * 48          # 2304 image positions
NJT = 18             # image key tiles of 128

CHUNKS = [(0, 512), (512, 512), (1024, 512), (1536, 512), (2048, 256)]
NCH = len(CHUNKS)
DVE_JTS = frozenset({2, 4, 7, 9, 11, 14, 16, 17})  # interleaved with ACT

A_SCH = 128.0 / math.log(2.0)        # bf16 Schraudolph slope
B_SCH = 127.0 * 128.0 - 5.5          # minimax-centered magic constant


def _jt_slice(jt):
    """key tile jt -> (chunk index, col offset within chunk)."""
    pos0 = jt * 128
    for ci, (c0, cw) in enumerate(CHUNKS):
        if c0 <= pos0 < c0 + cw:
            return ci, pos0 - c0
    raise AssertionError(jt)


def build():
    nc = bacc.Bacc("TRN2", target_bir_lowering=False, debug=False,
                   enable_asserts=True, num_devices=8)
    x_d = nc.dram_tensor("x", [DIM, N], F32, kind="ExternalInput").ap()
    wqkv_d = nc.dram_tensor("wqkv", [DIM, 384], F32, kind="ExternalInput").ap()
    memk_d = nc.dram_tensor("memk", [128, MEM], F32, kind="ExternalInput").ap()
    memvst_d = nc.dram_tensor("memvst", [MEM, 2, 128], F32,
                              kind="ExternalInput").ap()
    woutT_d = nc.dram_tensor("woutT", [2, DHEAD, DIM], F32,
                             kind="ExternalInput").ap()
    out_d = nc.dram_tensor("out", [2, 128, N], BF16,
                           kind="ExternalOutput").ap()

    with tile.TileContext(nc) as tc:
        with (
            tc.tile_pool(name="consts", bufs=1) as consts,
            tc.tile_pool(name="big", bufs=1) as big,
            tc.tile_pool(name="io", bufs=1) as io,
            tc.tile_pool(name="wk", bufs=2) as wk,
            tc.tile_pool(name="ps_s", bufs=2, space="PSUM") as ps_s,
            tc.tile_pool(name="ps_a", bufs=2, space="PSUM") as ps_a,
        ):
            # ---------------- input DMAs (sync queue, priority order) ------
            xs = [[None] * NCH, [None] * NCH]
            for ci, (c0, cw) in enumerate(CHUNKS):
                for kt in range(2):
                    xs[kt][ci] = big.tile([128, 512], F32, tag=f"x{kt}_{ci}",
                                          name=f"x{kt}_{ci}")
            wq_f = io.tile([128, 2, 384], F32, tag="wq_f")
            memk_f = io.tile([128, MEM], F32, tag="memk_f")
            memvst_f = io.tile([MEM, 2, 128], F32, tag="memv_f")
            woutA_f = io.tile([128, DIM], F32, tag="woutA_f")
            woutB_f = io.tile([128, DIM], F32, tag="woutB_f")

            nc.sync.dma_start(out=xs[0][0][:, :], in_=x_d[0:128, 0:512])
            nc.sync.dma_start(out=xs[1][0][:, :], in_=x_d[128:256, 0:512])
            nc.sync.dma_start(out=wq_f[:, 0, :], in_=wqkv_d[0:128, :])
            nc.sync.dma_start(out=wq_f[:, 1, :], in_=wqkv_d[128:256, :])
            for ci, (c0, cw) in enumerate(CHUNKS[1:], start=1):
                nc.sync.dma_start(out=xs[0][ci][:, 0:cw],
                                  in_=x_d[0:128, c0:c0 + cw])
                nc.sync.dma_start(out=xs[1][ci][:, 0:cw],
                                  in_=x_d[128:256, c0:c0 + cw])
            nc.sync.dma_start(out=memk_f[:, :], in_=memk_d)
            nc.sync.dma_start(out=memvst_f[:, :, :], in_=memvst_d)
            nc.sync.dma_start(out=woutA_f[64:128, :], in_=woutT_d[0, :, :])
            nc.gpsimd.dma_start(out=woutB_f[64:128, :], in_=woutT_d[1, :, :])

            # ---------------- constants / staging init --------------------
            ones_f = consts.tile([128, 1], F32)
            nc.vector.memset(ones_f[:, :], 1.0)
            ones_r = consts.tile([128, 128], F32R)
            nc.vector.tensor_copy(ones_r[:, :],
                                  ones_f[:, :].to_broadcast((128, 128)))

            # attnv lhsT staging: [keys, 2 heads, 128]
            # col 0 = 1.0 (denominator), 1:64 zeros, 64:128 = v^T
            vstag = [big.tile([128, 2, 128], BF16, tag=f"vst{jt}",
                              name=f"vst{jt}")
                     for jt in range(NJT + 1)]
            for jt in range(NJT):
                nc.vector.memset(vstag[jt][:, :, 0:1], 1.0)

            # weight conversion: bf16 qkv weights
            wq = consts.tile([128, 2, 384], BF16, tag="wq")
            nc.vector.tensor_copy(wq[:, :, :], wq_f[:, :, :])

            kb = [None] * NCH
            qb = [None] * NCH

            # ---------------- prep: per-chunk qkv --------------------------
            def prep_chunk(ci):
                c0, cw = CHUNKS[ci]
                njs = cw // 128
                xsq = wk.tile([128, 2, 512], F32R, tag="xsq", name=f"xsq{ci}")
                for kt in range(2):
                    nc.scalar.activation(xsq[:, kt, 0:cw],
                                         xs[kt][ci][:, 0:cw], SQUARE)
                ssq = ps_a.tile([128, 2, 512], F32, tag="a", name=f"ssq{ci}")
                for kt in range(2):
                    nc.tensor.matmul(ssq[:, 0, 0:cw], ones_r[:, :],
                                     xsq[:, kt, 0:cw],
                                     start=(kt == 0), stop=(kt == 1))
                # sigma rows = 16/l2 (sqrt then fast reciprocal)
                srow = wk.tile([128, 512], F32, tag="sr", name=f"sr{ci}")
                nc.scalar.activation(srow[:, 0:cw], ssq[:, 0, 0:cw], SQRT,
                                     scale=1.0 / 256.0)
                nc.vector.reciprocal_approx_fast(srow[:, 0:cw], srow[:, 0:cw])
                # xn = x * sigma (normalized x, bf16 for the matmuls)
                xn = wk.tile([128, 2, 512], BF16, tag="xn", name=f"xn{ci}")
                for kt in range(2):
                    nc.gpsimd.tensor_mul(xn[:, kt, 0:cw], xs[kt][ci][:, 0:cw],
                                         srow[:, 0:cw])
                # q, k projections
                qk = ps_s.tile([128, 2, 512], F32, tag="s", name=f"qk_{ci}")
                for m in range(2):  # 0 = q, 1 = k
                    for kt in range(2):
                        nc.tensor.matmul(
                            qk[:, m, 0:cw],
                            wq[:, kt, m * 128:m * 128 + 128],
                            xn[:, kt, 0:cw],
                            start=(kt == 0), stop=(kt == 1))
                qb[ci] = big.tile([128, 512], BF16, tag=f"q{ci}",
                                  name=f"qb{ci}")
                kb[ci] = big.tile([128, 512], BF16, tag=f"k{ci}",
                                  name=f"kb{ci}")
                nc.scalar.activation(qb[ci][:, 0:cw], qk[:, 0, 0:cw], COPY)
                nc.vector.tensor_copy(kb[ci][:, 0:cw], qk[:, 1, 0:cw])
                # v, directly transposed: [pos, 2 heads, 64]
                vps = ps_a.tile([128, 2, 4, 2, 64], F32, tag="a",
                                name=f"vps{ci}")
                for js in range(njs):
                    jt = ci * 4 + js
                    nc.gpsimd.memset(vstag[jt][:, :, 1:64], 0.0)
                    for kt in range(2):
                        nc.tensor.matmul(
                            vps[:, 0, js, :, :],
                            xn[:, kt, js * 128:js * 128 + 128],
                            wq[:, kt, 256:384],
                            start=(kt == 0), stop=(kt == 1))
                    nc.vector.tensor_copy(vstag[jt][:, :, 64:128],
                                          vps[:, 0, js, :, :])

            for ci in range(NCH):
                prep_chunk(ci)
            # late consts (off the critical path at start)
            kmem = consts.tile([128, MEM], BF16, tag="kmem")
            nc.vector.tensor_copy(kmem[:, :], memk_f[:, :])
            nc.vector.tensor_copy(vstag[NJT][0:MEM, :, :], memvst_f[:, :, :])
            woutA = consts.tile([128, DIM], BF16, tag="woutA")
            woutB = consts.tile([128, DIM], BF16, tag="woutB")
            nc.vector.tensor_copy(woutA[64:128, :], woutA_f[64:128, :])
            nc.vector.tensor_copy(woutB[64:128, :], woutB_f[64:128, :])
            wouts = [woutA, woutB]

            # ---------------- attention chunks ------------------------------
            accs_by_ci = [None] * NCH
            fin = {}

            def emit_sim(ci, jt):
                c0, cw = CHUNKS[ci]
                s_ps = ps_s.tile([128, 2, 512], F32, tag="s",
                                 name=f"s_{ci}_{jt}")
                if jt < NJT:
                    km = 128
                    kc, off = _jt_slice(jt)
                    klhs = [kb[kc][64 * h:64 * h + 64, off:off + 128]
                            for h in range(2)]
                else:
                    km = MEM
                    klhs = [kmem[64 * h:64 * h + 64, :] for h in range(2)]
                for h in range(2):
                    nc.tensor.matmul(
                        s_ps[0:km, h, 0:cw],
                        klhs[h],
                        qb[ci][64 * h:64 * h + 64, 0:cw],
                        start=True, stop=True)
                return s_ps, km

            def emit_exp(ci, jt, s_ps, km):
                c0, cw = CHUNKS[ci]
                if jt in DVE_JTS:
                    P = wk.tile([128, 2, 512], U16, tag="Pd",
                                name=f"Pd_{ci}_{jt}")
                    nc.vector.tensor_scalar(
                        out=P[:, :, 0:cw], in0=s_ps[:, :, 0:cw],
                        scalar1=A_SCH, scalar2=B_SCH,
                        op0=MULT, op1=ADD)
                    return P, True
                P = wk.tile([128, 2, 512], BF16, tag="Pa", name=f"Pa_{ci}_{jt}")
                nc.scalar.activation(P[0:km, :, 0:cw], s_ps[0:km, :, 0:cw],
                                     EXP)
                return P, False

            def emit_attnv(ci, jt, P, km, cast):
                c0, cw = CHUNKS[ci]
                acc = accs_by_ci[ci]
                for h in range(2):
                    rhs = P[0:km, h, 0:cw]
                    if cast:
                        rhs = rhs.bitcast(BF16)
                    nc.tensor.matmul(
                        acc[:, h, 0:cw],
                        vstag[jt][0:km, h, :],
                        rhs,
                        start=(jt == 0), stop=(jt == NJT),
                        skip_group_check=True)

            def emit_fin_a(ci):
                """den reciprocal + partition broadcast."""
                c0, cw = CHUNKS[ci]
                acc = accs_by_ci[ci]
                rec = wk.tile([128, 2, 512], F32, tag="rec", name=f"rec{ci}")
                rb = wk.tile([128, 2, 512], F32, tag="rb", name=f"rb{ci}")
                nc.vector.reciprocal_approx_fast(rec[0:1, :, 0:cw],
                                                 acc[0:1, :, 0:cw])
                nc.gpsimd.partition_broadcast(rb[:, :, 0:cw],
                                              rec[0:1, :, 0:cw])
                fin[ci] = rb

            def emit_fin_b(ci):
                """normalize oT (DVE)."""
                c0, cw = CHUNKS[ci]
                acc = accs_by_ci[ci]
                rb = fin.pop(ci)
                oT = wk.tile([128, 2, 512], BF16, tag="oT", name=f"oT_{ci}")
                nc.vector.tensor_mul(oT[64:128, :, 0:cw],
                                     acc[64:128, :, 0:cw],
                                     rb[64:128, :, 0:cw])
                fin[ci] = oT

            def emit_fin_c(ci):
                """partial out-projection (PE)."""
                c0, cw = CHUNKS[ci]
                oT = fin.pop(ci)
                op = ps_s.tile([128, 2, 512], F32, tag="s", name=f"op_{ci}")
                for mt in range(2):
                    for h in range(2):
                        nc.tensor.matmul(
                            op[:, mt, 0:cw],
                            wouts[h][64:128, mt * 128:mt * 128 + 128],
                            oT[64:128, h, 0:cw],
                            start=(h == 0), stop=(h == 1))
                fin[ci] = op

            def emit_fin_d(ci):
                """osb cast (ACT) + output DMA (sync)."""
                c0, cw = CHUNKS[ci]
                op = fin.pop(ci)
                osb = wk.tile([128, 2, 512], BF16, tag="osb", name=f"osb{ci}")
                nc.scalar.activation(osb[:, :, 0:cw], op[:, :, 0:cw], COPY)
                for mt in range(2):
                    nc.sync.dma_start(out=out_d[mt, :, c0:c0 + cw],
                                      in_=osb[:, mt, 0:cw])

            def hooks(ci, jt):
                if ci == 0:
                    return
                if jt == 3:
                    emit_fin_a(ci - 1)
                elif jt == 5:
                    emit_fin_b(ci - 1)
                elif jt == 8:
                    emit_fin_c(ci - 1)
                elif jt == 10:
                    emit_fin_d(ci - 1)

            for ci, (c0, cw) in enumerate(CHUNKS):
                accs_by_ci[ci] = ps_a.tile([128, 2, 512], F32, tag="a",
                                           name=f"acc_{ci}")
                pend = None
                for jt in range(NJT + 1):
                    s_ps, km = emit_sim(ci, jt)
                    if pend is not None:
                        emit_attnv(ci, *pend)
                    hooks(ci, jt)
                    P, cast = emit_exp(ci, jt, s_ps, km)
                    pend = (jt, P, km, cast)
                emit_attnv(ci, *pend)
            emit_fin_a(NCH - 1)
            emit_fin_b(NCH - 1)
            emit_fin_c(NCH - 1)
            emit_fin_d(NCH - 1)
    nc.compile()
    return nc


_NC = None
_last_in_maps = None


def _get_nc():
    global _NC
    if _NC is None:
        _NC = build()
    return _NC


def make_in_maps(x, gamma, mem_kv, w_qkv, w_out):
    x = np.asarray(x, np.float32)
    gamma = np.asarray(gamma, np.float32).reshape(DIM)
    mem_kv = np.asarray(mem_kv, np.float32)
    w_qkv = np.asarray(w_qkv, np.float32)
    w_out = np.asarray(w_out, np.float32)

    g1 = 1.0 + gamma  # [256]
    scale = DHEAD ** -0.5
    in_maps = []
    for core in range(8):
        b, g = core // 4, core % 4
        hA, hB = 2 * g, 2 * g + 1
        blocks = []
        for t in range(3):  # q, k, v
            for h in (hA, hB):
                wblk = w_qkv[t * HID + h * DHEAD: t * HID + (h + 1) * DHEAD, :]
                if t == 0:
                    wblk = wblk * scale
                blocks.append(wblk.T)  # [256, 64]
        wqkvT = np.concatenate(blocks, axis=1) * g1[:, None]  # [256, 384]
        memk = np.concatenate(
            [mem_kv[0, hA].T, mem_kv[0, hB].T], axis=0)  # [128, 4]
        # mem staging: [4, 2, 128] = [1.0 (den) | zeros | v (64:128)]
        memvst = np.zeros((MEM, 2, 128), np.float32)
        memvst[:, :, 0] = 1.0
        memvst[:, 0, 64:128] = mem_kv[1, hA]
        memvst[:, 1, 64:128] = mem_kv[1, hB]
        # wout rows: head A weights at partitions 64:128, head B at 0:64
        woutT = np.stack(
            [w_out[:, hA * DHEAD:(hA + 1) * DHEAD].T,
             w_out[:, hB * DHEAD:(hB + 1) * DHEAD].T], axis=0)  # [2, 64, 256]
        in_maps.append({
            "x": np.ascontiguousarray(x[b].reshape(DIM, N)),
            "wqkv": np.ascontiguousarray(wqkvT),
            "memk": np.ascontiguousarray(memk),
            "memvst": np.ascontiguousarray(memvst),
            "woutT": np.ascontiguousarray(woutT),
        })
    return in_maps


def kernel(x, gamma, mem_kv, w_qkv, w_out):
    global _last_in_maps
    in_maps = make_in_maps(x, gamma, mem_kv, w_qkv, w_out)
    _last_in_maps = in_maps
    nc = _get_nc()
    res = run_bass_kernel_spmd(nc, in_maps, core_ids=list(range(8)))
    out = np.zeros((2, DIM, N), np.float32)
    for core in range(8):
        b = core // 4
        part = np.asarray(res.results[core]["out"], dtype=np.float32)
        out[b, 0:128, :] += part[0]
        out[b, 128:256, :] += part[1]
    return out.reshape(2, DIM, 48, 48)
